# revision 6
# baseline (speedup 1.0000x reference)
"""Trainium2 Bass kernel for nn_KnowledgeInjectionDeepfakeDetector.

Reference computation (B=16, N=1024, D=768, H=12, hd=64):
    q,k,v,qt = split_heads(f @ W_* + b_*)
    corr   = qt @ k^T / 8                                  [B,H,N,N]
    attn   = softmax(q @ k^T / 8 + corr) @ v  -> [B,N,D]
    fn     = LN(loc + pos)
    loc    = softmax(mean_h corr) @ fn @ W_kloc + b_kloc    [B,N,D]

Key algebraic rewrites (validated vs reference in fp64/fp32 numpy):
  * scores = (q + qt) @ k^T / 8  -> fold (W_q+W_qt)/8 on host, one matmul.
  * mean_h corr = (qt_full . k_full) / (8*12): the head-mean collapses into a
    single K=768 matmul of the *unsplit* projections.
  * softmax without max-subtraction (scores std ~0.45, |max| < 4 -> exp safe).
  * Scores computed transposed (S^T[m,n], keys on partitions) so that
    O'^T = v'^T @ E^T needs no attention-matrix transpose;  v' carries an
    extra ones column so row 64 of O'^T is the softmax denominator.
  * loc: exp(meanT) kept transposed; t'^T = fn^T @ E_mean^T; the softmax
    normalizer is applied to the final [n,:] rows as a per-partition scale.

Sharding: pure data-parallel, 2 batch elements per NeuronCore across 8 cores.
"""

import numpy as np

import concourse.bacc as bacc
import concourse.bass as bass
import concourse.tile as tile
from concourse import mybir
from concourse.bass import ts
from concourse.bass_utils import run_bass_kernel_spmd
from concourse.masks import make_identity

F32 = mybir.dt.float32
AF = mybir.ActivationFunctionType
ALU = mybir.AluOpType
AX = mybir.AxisListType

B, N, D, H, HD = 16, 1024, 768, 12, 64
SCALE = 8.0                # sqrt(64)
NCORES = 8
BB = B // NCORES           # batches per core
NT = N // 128              # 8 n-tiles
DC = D // 128              # 6 d-chunks
EPS = 1e-5


def _build(trace_mode=False):
    nc = bacc.Bacc("TRN2", target_bir_lowering=False, debug=False,
                   enable_asserts=False)

    # ---- DRAM I/O (per-core shard) ----
    f_d = nc.dram_tensor("features", [BB, N, D], F32, kind="ExternalInput").ap()
    lf_d = nc.dram_tensor("loc_features", [BB, N, D], F32, kind="ExternalInput").ap()
    wqts_d = nc.dram_tensor("w_qts", [D, D], F32, kind="ExternalInput").ap()
    wqs_d = nc.dram_tensor("w_qsum", [D, D], F32, kind="ExternalInput").ap()
    wk_d = nc.dram_tensor("w_k", [D, D], F32, kind="ExternalInput").ap()
    wv_d = nc.dram_tensor("w_v", [D, D], F32, kind="ExternalInput").ap()
    wkl_d = nc.dram_tensor("w_kloc", [D, D], F32, kind="ExternalInput").ap()
    bqts_d = nc.dram_tensor("b_qts", [D], F32, kind="ExternalInput").ap()
    bqs_d = nc.dram_tensor("b_qsum", [D], F32, kind="ExternalInput").ap()
    bk_d = nc.dram_tensor("b_k", [D], F32, kind="ExternalInput").ap()
    bv_d = nc.dram_tensor("b_v", [D], F32, kind="ExternalInput").ap()
    bkl_d = nc.dram_tensor("b_kloc", [D], F32, kind="ExternalInput").ap()
    pos_d = nc.dram_tensor("pos_enc", [N, D], F32, kind="ExternalInput").ap()
    gam_d = nc.dram_tensor("ln_gamma", [D], F32, kind="ExternalInput").ap()
    bet_d = nc.dram_tensor("ln_beta", [D], F32, kind="ExternalInput").ap()

    attn_d = nc.dram_tensor("attn_out", [BB, N, D], F32, kind="ExternalOutput").ap()
    loc_d = nc.dram_tensor("loc_out", [BB, N, D], F32, kind="ExternalOutput").ap()
    corr_d = nc.dram_tensor("corr", [BB, H, N, N], F32, kind="ExternalOutput").ap()

    with tile.TileContext(nc) as tc:
        _body(nc, tc, f_d, lf_d, wqts_d, wqs_d, wk_d, wv_d, wkl_d,
              bqts_d, bqs_d, bk_d, bv_d, bkl_d, pos_d, gam_d, bet_d,
              attn_d, loc_d, corr_d)
    nc.compile()
    return nc


def _body(nc, tc, f_d, lf_d, wqts_d, wqs_d, wk_d, wv_d, wkl_d,
          bqts_d, bqs_d, bk_d, bv_d, bkl_d, pos_d, gam_d, bet_d,
          attn_d, loc_d, corr_d):
    from contextlib import ExitStack

    # ---------------- persistent constants ----------------
    const = tc.alloc_tile_pool(name="const", bufs=1)
    ident = const.tile([128, 128], F32)
    make_identity(nc, ident)
    eps_t = const.tile([128, 1], F32)
    nc.vector.memset(eps_t, EPS)
    ones_row = const.tile([1, 128], F32)
    nc.vector.memset(ones_row, 1.0)
    ones_col = const.tile([128, 1], F32)
    nc.vector.memset(ones_col, 1.0)

    # per-partition bias columns for the transposed projections: [128, DC]
    bqts_t = const.tile([128, DC], F32)
    nc.sync.dma_start(out=bqts_t, in_=bqts_d.rearrange("(c p) -> p c", p=128))
    bqs_t = const.tile([128, DC], F32)
    nc.sync.dma_start(out=bqs_t, in_=bqs_d.rearrange("(c p) -> p c", p=128))
    bk_t = const.tile([128, DC], F32)
    nc.sync.dma_start(out=bk_t, in_=bk_d.rearrange("(c p) -> p c", p=128))

    # broadcast rows -> [128, D] tiles via PE (ones_row^T @ row)
    bv_row = const.tile([1, D], F32)
    nc.sync.dma_start(out=bv_row, in_=bv_d.rearrange("(a d) -> a d", a=1))
    gam_row = const.tile([1, D], F32)
    nc.sync.dma_start(out=gam_row, in_=gam_d.rearrange("(a d) -> a d", a=1))
    bet_row = const.tile([1, D], F32)
    nc.sync.dma_start(out=bet_row, in_=bet_d.rearrange("(a d) -> a d", a=1))
    bkl_row = const.tile([1, D], F32)
    nc.sync.dma_start(out=bkl_row, in_=bkl_d.rearrange("(a d) -> a d", a=1))

    bv_bc = const.tile([128, D], F32)
    gam_bc = const.tile([128, D], F32)
    bet_bc = const.tile([128, D], F32)
    bkl_bc = const.tile([128, D], F32)
    with tc.tile_pool(name="bcastp", bufs=2, space="PSUM") as bcp:
        for row, bc in ((bv_row, bv_bc), (gam_row, gam_bc),
                        (bet_row, bet_bc), (bkl_row, bkl_bc)):
            ps = bcp.tile([128, D], F32, tag="bc")
            for w0, wn in ((0, 512), (512, 256)):
                nc.tensor.matmul(ps[:, w0:w0 + wn], lhsT=ones_row,
                                 rhs=row[:, w0:w0 + wn], start=True, stop=True)
            nc.vector.tensor_copy(bc, ps)

    # ---------------- long-lived big tensors ----------------
    big = tc.alloc_tile_pool(name="big", bufs=1)

    for b in range(BB):
        # ================= PHASE P : f^T + projections =================
        with ExitStack() as st:
            fnat_p = st.enter_context(tc.tile_pool(name="fnat_p", bufs=3))
            w_p = st.enter_context(tc.tile_pool(name="w_p", bufs=2))
            pp = st.enter_context(tc.tile_pool(name="pp", bufs=3, space="PSUM"))
            ptr = st.enter_context(tc.tile_pool(name="ptr", bufs=2, space="PSUM"))

            fT = big.tile([128, DC, N], F32, tag="fx")      # f^T  [d, n]
            for i in range(NT):
                fnat = fnat_p.tile([128, D], F32, tag="fnat")
                nc.sync.dma_start(out=fnat, in_=f_d[b, ts(i, 128), :])
                for c in range(DC):
                    tp = ptr.tile([128, 128], F32, tag="tr")
                    nc.tensor.transpose(tp, fnat[:, ts(c, 128)], ident)
                    if (i * DC + c) % 2 == 0:
                        nc.vector.tensor_copy(fT[:, c, ts(i, 128)], tp)
                    else:
                        nc.scalar.copy(fT[:, c, ts(i, 128)], tp)

            # transposed projections: out^T[d, n] = W^T @ f^T
            qtsT = big.tile([128, DC, N], F32, tag="qtsT")
            qsT = big.tile([128, DC, N], F32, tag="qsT")
            kT = big.tile([128, DC, N], F32, tag="kT")
            for pi, (w_dram, bias_t, outT) in enumerate((
                    (wqts_d, bqts_t, qtsT),
                    (wqs_d, bqs_t, qsT),
                    (wk_d, bk_t, kT))):
                wsb = w_p.tile([128, DC, D], F32, tag="w")
                nc.sync.dma_start(out=wsb, in_=w_dram.rearrange("(c p) m -> p c m", p=128))
                for mc in range(DC):
                    ps = pp.tile([128, N], F32, tag="proj")
                    for half in range(2):
                        for kc in range(DC):
                            nc.tensor.matmul(
                                ps[:, ts(half, 512)],
                                lhsT=wsb[:, kc, ts(mc, 128)],
                                rhs=fT[:, kc, ts(half, 512)],
                                start=(kc == 0), stop=(kc == DC - 1))
                    if (pi * DC + mc) % 2 == 0:
                        nc.vector.tensor_scalar_add(outT[:, mc, :], ps,
                                                    bias_t[:, mc:mc + 1])
                    else:
                        nc.scalar.activation(outT[:, mc, :], ps, AF.Identity,
                                             bias=bias_t[:, mc:mc + 1])

            # v' natural [n, H, 65] with ones column
            vp = big.tile([128, NT, H, HD + 1], F32, tag="vp")
            wsb = w_p.tile([128, DC, D], F32, tag="w")
            nc.sync.dma_start(out=wsb, in_=wv_d.rearrange("(c p) m -> p c m", p=128))
            for i in range(NT):
                ps = pp.tile([128, D], F32, tag="proj")
                for w0, wn in ((0, 512), (512, 256)):
                    for kc in range(DC):
                        nc.tensor.matmul(
                            ps[:, w0:w0 + wn],
                            lhsT=fT[:, kc, ts(i, 128)],
                            rhs=wsb[:, kc, w0:w0 + wn],
                            start=(kc == 0), stop=(kc == DC - 1))
                nc.vector.tensor_tensor(
                    out=vp[:, i, :, 0:HD],
                    in0=ps.rearrange("p (h c) -> p h c", c=HD),
                    in1=bv_bc.rearrange("p (h c) -> p h c", c=HD), op=ALU.add)
            nc.vector.memset(vp[:, :, :, HD:HD + 1], 1.0)

        # ================= PHASE A : attention + corr =================
        with ExitStack() as st:
            ps_s = st.enter_context(tc.tile_pool(name="ps_s", bufs=2, space="PSUM"))
            ps_c = st.enter_context(tc.tile_pool(name="ps_c", bufs=2, space="PSUM"))
            ps_o = st.enter_context(tc.tile_pool(name="ps_o", bufs=2, space="PSUM"))
            sb_c = st.enter_context(tc.tile_pool(name="sb_c", bufs=3))
            sb_e = st.enter_context(tc.tile_pool(name="sb_e", bufs=3))
            sb_o = st.enter_context(tc.tile_pool(name="sb_o", bufs=2))
            sb_r = st.enter_context(tc.tile_pool(name="sb_r", bufs=4))

            for h in range(H):
                kc, off = h // 2, (h % 2) * HD
                khead = kT[off:off + HD, kc, :]      # [64, N] head slice
                # ---- corr (natural layout) -> DRAM ----
                for i in range(NT):
                    csb = sb_c.tile([128, N], F32, tag="csb")
                    for half in range(2):
                        cps = ps_c.tile([128, 512], F32, tag="corr")
                        nc.tensor.matmul(
                            cps,
                            lhsT=qtsT[off:off + HD, kc, ts(i, 128)],
                            rhs=khead[:, ts(half, 512)],
                            start=True, stop=True)
                        if (i + half) % 2 == 0:
                            nc.vector.tensor_copy(csb[:, ts(half, 512)], cps)
                        else:
                            nc.scalar.copy(csb[:, ts(half, 512)], cps)
                    nc.sync.dma_start(out=corr_d[b, h, ts(i, 128), :], in_=csb)

                # ---- S^T -> exp -> O'^T accumulate ----
                op0 = ps_o.tile([HD + 1, 512], F32, tag="op")
                op1 = ps_o.tile([HD + 1, 512], F32, tag="op")
                for m in range(NT):
                    sps = ps_s.tile([128, N], F32, tag="s")
                    for half in range(2):
                        nc.tensor.matmul(
                            sps[:, ts(half, 512)],
                            lhsT=khead[:, ts(m, 128)],
                            rhs=qsT[off:off + HD, kc, ts(half, 512)],
                            start=True, stop=True)
                    et = sb_e.tile([128, N], F32, tag="E")
                    nc.scalar.activation(et, sps, AF.Exp)
                    vh = vp[:, m, h, :]              # [128, 65]
                    nc.tensor.matmul(op0, lhsT=vh, rhs=et[:, 0:512],
                                     start=(m == 0), stop=(m == NT - 1))
                    nc.tensor.matmul(op1, lhsT=vh, rhs=et[:, 512:1024],
                                     start=(m == 0), stop=(m == NT - 1))

                # ---- evacuate O', transpose, normalize, store ----
                osb = sb_o.tile([HD + 1, N], F32, tag="osb")
                nc.vector.tensor_copy(osb[:, 0:512], op0)
                nc.vector.tensor_copy(osb[:, 512:1024], op1)
                oh = sb_o.tile([128, NT, HD], F32, tag="oh")
                for i in range(NT):
                    tp = ps_c.tile([128, HD + 1], F32, tag="corr")
                    nc.tensor.transpose(tp, osb[:, ts(i, 128)],
                                        ident[0:HD + 1, 0:HD + 1])
                    rc = sb_r.tile([128, 1], F32, tag="rc")
                    nc.vector.reciprocal(rc, tp[:, HD:HD + 1])
                    nc.vector.tensor_scalar_mul(oh[:, i, :], tp[:, 0:HD], rc)
                nc.sync.dma_start(
                    out=attn_d[b].rearrange("(i p) (h c) -> p i h c",
                                            p=128, c=HD)[:, :, h, :],
                    in_=oh)

        # ================= PHASE L : localization branch =================
        with ExitStack() as st:
            sb_f = st.enter_context(tc.tile_pool(name="sb_f", bufs=3))
            sb_m = st.enter_context(tc.tile_pool(name="sb_m", bufs=3))
            sb_d = st.enter_context(tc.tile_pool(name="sb_d", bufs=1))
            fn = big.tile([128, NT, D], F32, tag="fx")       # reuses f^T slot
            tpr = big.tile([128, DC, N], F32, tag="qsT")     # reuses qs^T slot

            # LN(loc + pos)
            for i in range(NT):
                fni = fn[:, i, :]
                nc.sync.dma_start(out=fni, in_=lf_d[b, ts(i, 128), :])
                pos_t = sb_f.tile([128, D], F32, tag="pos")
                nc.sync.dma_start(out=pos_t, in_=pos_d[ts(i, 128), :])
                nc.vector.tensor_tensor(out=fni, in0=fni, in1=pos_t, op=ALU.add)
                stats = sb_f.tile([128, 3, 6], F32, tag="stats")
                for sg in range(3):
                    nc.vector.bn_stats(out=stats[:, sg, :],
                                       in_=fni[:, ts(sg, 256)])
                mv = sb_f.tile([128, 2], F32, tag="mv")
                nc.vector.bn_aggr(out=mv, in_=stats)
                std = sb_f.tile([128, 1], F32, tag="std")
                nc.scalar.activation(std, mv[:, 1:2], AF.Sqrt, bias=eps_t)
                rstd = sb_f.tile([128, 1], F32, tag="rstd")
                nc.vector.reciprocal(rstd, std)
                nc.vector.tensor_scalar(out=fni, in0=fni, scalar1=mv[:, 0:1],
                                        scalar2=None, op0=ALU.subtract)
                nc.vector.scalar_tensor_tensor(out=fni, in0=fni, scalar=rstd,
                                               in1=gam_bc, op0=ALU.mult,
                                               op1=ALU.mult)
                nc.vector.tensor_tensor(out=fni, in0=fni, in1=bet_bc, op=ALU.add)

            # meanT -> exp -> t'^T accumulation (+ softmax denominator)
            den_sb = sb_d.tile([1, N], F32)
            with ExitStack() as st2:
                ps_t = st2.enter_context(tc.tile_pool(name="ps_t", bufs=1, space="PSUM"))
                ps_mn = st2.enter_context(tc.tile_pool(name="ps_mn", bufs=1, space="PSUM"))
                ps_dn = st2.enter_context(tc.tile_pool(name="ps_dn", bufs=1, space="PSUM"))
                for half in range(2):
                    tacc = [ps_t.tile([128, 512], F32, tag=f"t{mc}", bufs=1,
                                      name=f"tacc{mc}")
                            for mc in range(DC)]
                    dn = ps_dn.tile([1, 512], F32, tag="dn")
                    for m in range(NT):
                        mps = ps_mn.tile([128, 512], F32, tag="mean")
                        for kc in range(DC):
                            nc.tensor.matmul(
                                mps,
                                lhsT=kT[:, kc, ts(m, 128)],
                                rhs=qtsT[:, kc, ts(half, 512)],
                                start=(kc == 0), stop=(kc == DC - 1))
                        em = sb_m.tile([128, 512], F32, tag="em")
                        nc.scalar.activation(em, mps, AF.Exp, scale=1.0 / H)
                        nc.tensor.matmul(dn, lhsT=ones_col, rhs=em,
                                         start=(m == 0), stop=(m == NT - 1))
                        for mc in range(DC):
                            nc.tensor.matmul(
                                tacc[mc],
                                lhsT=fn[:, m, ts(mc, 128)],
                                rhs=em,
                                start=(m == 0), stop=(m == NT - 1))
                    for mc in range(DC):
                        if mc % 2 == 0:
                            nc.vector.tensor_copy(tpr[:, mc, ts(half, 512)], tacc[mc])
                        else:
                            nc.scalar.copy(tpr[:, mc, ts(half, 512)], tacc[mc])
                    nc.scalar.copy(den_sb[:, ts(half, 512)], dn)

            # r[n] column form + out2 = r * (t' @ W_kloc) + b_kloc
            with ExitStack() as st2:
                ps_o2 = st2.enter_context(tc.tile_pool(name="ps_o2", bufs=2, space="PSUM"))
                ps_dt = st2.enter_context(tc.tile_pool(name="ps_dt", bufs=2, space="PSUM"))
                rl = sb_d.tile([128, NT], F32)
                for i in range(NT):
                    dtp = ps_dt.tile([128, 1], F32, tag="dtr")
                    nc.tensor.transpose(dtp, den_sb[:, ts(i, 128)], ident[0:1, 0:1])
                    nc.vector.reciprocal(rl[:, i:i + 1], dtp)

                wsb = sb_m.tile([128, DC, D], F32, tag="wkl", bufs=1)
                nc.sync.dma_start(out=wsb, in_=wkl_d.rearrange("(c p) m -> p c m", p=128))
                for i in range(NT):
                    o2 = ps_o2.tile([128, D], F32, tag="o2")
                    for w0, wn in ((0, 512), (512, 256)):
                        for kc in range(DC):
                            nc.tensor.matmul(
                                o2[:, w0:w0 + wn],
                                lhsT=tpr[:, kc, ts(i, 128)],
                                rhs=wsb[:, kc, w0:w0 + wn],
                                start=(kc == 0), stop=(kc == DC - 1))
                    lout = sb_m.tile([128, D], F32, tag="lout", bufs=2)
                    nc.vector.scalar_tensor_tensor(
                        out=lout, in0=o2, scalar=rl[:, i:i + 1], in1=bkl_bc,
                        op0=ALU.mult, op1=ALU.add)
                    nc.sync.dma_start(out=loc_d[b, ts(i, 128), :], in_=lout)

    big.release()
    const.release()


_NC_CACHE = {}


def _get_nc():
    if "nc" not in _NC_CACHE:
        _NC_CACHE["nc"] = _build()
    return _NC_CACHE["nc"]


def _prep_host(inputs):
    """Host-side weight folding; returns dict of per-core-shared tensors."""
    g = {k: np.asarray(v, dtype=np.float32) for k, v in inputs.items()}
    return {
        "w_qts": np.ascontiguousarray(g["w_q_tilde"] / SCALE),
        "b_qts": np.ascontiguousarray(g["b_q_tilde"] / SCALE),
        "w_qsum": np.ascontiguousarray((g["w_q"] + g["w_q_tilde"]) / SCALE),
        "b_qsum": np.ascontiguousarray((g["b_q"] + g["b_q_tilde"]) / SCALE),
        "w_k": g["w_k"], "b_k": g["b_k"],
        "w_v": g["w_v"], "b_v": g["b_v"],
        "w_kloc": g["w_k_loc"], "b_kloc": g["b_k_loc"],
        "pos_enc": np.ascontiguousarray(g["pos_enc"].reshape(N, D)),
        "ln_gamma": g["ln_gamma"], "ln_beta": g["ln_beta"],
    }, g


def kernel(**inputs):
    shared, g = _prep_host(inputs)
    feats = np.ascontiguousarray(g["features"])
    locf = np.ascontiguousarray(g["loc_features"])

    in_maps = []
    for c in range(NCORES):
        m = dict(shared)
        m["features"] = np.ascontiguousarray(feats[c * BB:(c + 1) * BB])
        m["loc_features"] = np.ascontiguousarray(locf[c * BB:(c + 1) * BB])
        in_maps.append(m)

    nc = _get_nc()
    res = run_bass_kernel_spmd(nc, in_maps, list(range(NCORES)))
    outs = res.results

    attn = np.concatenate([outs[c]["attn_out"] for c in range(NCORES)], axis=0)
    loc = np.concatenate([outs[c]["loc_out"] for c in range(NCORES)], axis=0)
    corr = np.concatenate([outs[c]["corr"] for c in range(NCORES)], axis=0)
    return attn, loc, corr
